# revision 7
# baseline (speedup 1.0000x reference)
"""AdaptiveQuantizer Trainium2 kernel (8 NeuronCores, data-parallel over batch).

Math (per pixel (b,h,w), over C=64 channels):
    fmin/fmax = min/max over channels
    rng  = fmax - fmin (+1e-30 guard)
    lm1  = 2**bits - 1                (exact, via int shift trick)
    u    = lm1 / rng
    c2   = -u*fmin
    w    = u*f + c2                   in [0, lm1]
    r    = round_half_even(w)         via fp32 +M / -M (M = 1.5*2**23)
    out  = (rng/lm1)*r + fmin

Perf design (cost-model driven):
  * DVE tensor_tensor is 1x for f32 and tensor_reduce is ALWAYS 1x, but
    TensorScalarPtr (scalar_tensor_tensor / tensor_scalar) runs 2x for
    f32-in-SBUF and 4x when every tensor operand is packed 16-bit.
  * Channel min/max therefore uses a binary TREE of 2x f32 stt max/min ops
    (4.2us/stat/superblock) instead of 1x strided tensor_reduce (8.5us).
  * The two f32 elementwise passes (mult-u, add-c2) are in-place stts (2x).
  * Rounding runs on the otherwise-idle ACT engine: pass1 +M (f32), pass2
    -M writing bf16 (r <= 255 is EXACT in bf16).
  * The post-round tail ((r*v)+fmin) is all-bf16 stts at 4x.
  * Emission is software-pipelined (front(i+1) before back(i)) so DVE chews
    superblock i+1's tree while ACT rounds superblock i.
"""

import os
import sys
from contextlib import nullcontext

for _p in ("/opt/trn_rl_repo", "/root/.axon_site/_ro/trn_rl_repo"):
    if os.path.isdir(_p) and _p not in sys.path:
        sys.path.insert(0, _p)

import numpy as np

import concourse.bass as bass
import concourse.mybir as mybir
from concourse.bass_utils import run_bass_kernel_spmd
from concourse.tile import TileContext
from concourse.vector_clock import ScopedClock

# Problem shapes (hardcoded per spec)
B_FULL, C, H, W = 16, 64, 256, 256
N_CORES = 8
B_LOC = B_FULL // N_CORES  # images per core
PX = H * W                 # pixels per image
P = 128                    # SBUF partitions
WPP = int(os.environ.get("KWPP", "128"))   # pixels per partition per superblock
F_BUFS = int(os.environ.get("KFBUFS", "3"))
FB_BUFS = int(os.environ.get("KFBBUFS", "2"))
CCH = int(os.environ.get("KCCH", "16"))    # channels per DMA chunk
M_MAGIC = 12582912.0       # 1.5*2**23: fp32 "+M" add == round-to-nearest-even
AL = mybir.AluOpType
F32 = mybir.dt.float32
I32 = mybir.dt.int32
BF16 = mybir.dt.bfloat16

_drain_patched = False


def _patch_tile_drain():
    """This container's walrus accepts only ONE sync wait per TPB_CTRL
    instruction; Tile's final drain carries one wait per ticked proc.
    Split them across multiple drains."""
    global _drain_patched
    if _drain_patched:
        return
    _drain_patched = True

    def _patched(self, tick_clock, wait_clock):
        nc = self.nc
        drain_inst = nc.sync.drain()
        wait_clock.add_sem_waits(
            drain_inst.ins, ScopedClock({None: tick_clock.global_clock})
        )
        si = drain_inst.ins.sync_info
        waits = list(si.on_wait) if (si is not None and si.on_wait) else []
        if len(waits) > 1:
            si.on_wait = waits[:1]
            for wchunk in waits[1:]:
                extra = nc.sync.drain()
                esi = extra.ins.sync_info
                if esi is None:
                    extra.ins.sync_info = mybir.SyncInfo(
                        on_wait=[wchunk], on_update=[]
                    )
                else:
                    esi.on_wait = [wchunk]
        nc.all_engine_barrier()
        assert self.sems is not None
        popped = nc._tile_sem_poison_stack.pop()
        assert popped is self._sem_poison
        nc.clear_and_free_semaphores(list(self.sems.allocated().values()))
        nc.all_engine_barrier()

    TileContext._drain_and_barrier = _patched


def _split_sync_waits(nc: bass.Bass, max_waits: int = 1) -> None:
    """This container's walrus rejects instructions carrying more than one
    sync wait. Hoist excess waits onto injected same-engine NOPs placed
    immediately before the instruction (engine program order makes this
    semantically identical)."""
    k = 0
    for bb in nc.main_func.blocks:
        insts = list(bb.instructions)
        out_list = []
        changed = False
        for inst in insts:
            si = inst.sync_info
            waits = list(si.on_wait) if (si is not None and si.on_wait) else []
            if len(waits) > max_waits:
                keep = waits[-max_waits:]
                hoist = waits[:-max_waits]
                for i in range(0, len(hoist), max_waits):
                    nop = mybir.InstNoOp(name=f"WSPL-{k}", ins=[], outs=[])
                    k += 1
                    nop.engine = inst.engine
                    nop.sync_info = mybir.SyncInfo(
                        on_wait=hoist[i : i + max_waits], on_update=[]
                    )
                    out_list.append(nop)
                si.on_wait = keep
                changed = True
            out_list.append(inst)
        if changed:
            bb.instructions.clear()
            for inst in out_list:
                bb.instructions.append(inst)


def _minmax_tree(nc, spool, Fv, out_t, op, wpp):
    """Per-pixel channel-{max,min} via a binary tree of 2x f32 stt ops.
    Fv: [P, C, wpp] view of the loaded superblock. out_t: [P, wpp] tile."""
    sA = spool.tile([P, (C // 2) * wpp], F32, tag="sA", bufs=1)
    sB = spool.tile([P, (C // 4) * wpp], F32, tag="sB", bufs=1)
    vA = sA[:].rearrange("p (c w) -> p c w", c=C // 2)
    vB = sB[:].rearrange("p (c w) -> p c w", c=C // 4)
    cur = Fv
    nch = C
    views = [vA, vB]
    bi = 0
    while nch > 4:
        half = nch // 2
        dst = views[bi % 2][:, :half, :]
        nc.vector.scalar_tensor_tensor(
            dst, cur[:, :half, :], 0.0, cur[:, half:nch, :], AL.add, op
        )
        cur = dst
        nch = half
        bi += 1
    # fold the last 4 channels with one strided reduce (saves 2 op overheads)
    nc.vector.tensor_reduce(
        out_t[:],
        cur[:, 0:4, :].rearrange("p c w -> p w c"),
        axis=mybir.AxisListType.X,
        op=op,
    )


def _front(nc, fpool, ppool, spool, feat, bits, b, s, do_red, do_elem, wpp,
           f_bufs, fb_bufs):
    """Load + stats + params + pre-round elementwise + ACT rounding.
    Returns state needed by _back()."""
    SB_PX = P * wpp
    px0 = s * SB_PX
    F = fpool.tile([P, C * wpp], F32, tag="F", bufs=f_bufs)
    Fv = F[:].rearrange("p (c w) -> p c w", c=C)
    # ---- load: 1 MiB chunks, contiguous 512 B runs ----
    for cc in range(0, C, CCH):
        src = feat[b, cc : cc + CCH, px0 : px0 + SB_PX]
        src = src.rearrange("c (p w) -> p c w", p=P)
        nc.sync.dma_start(out=Fv[:, cc : cc + CCH, :], in_=src)
    bt = ppool.tile([P, wpp], I32, tag="bt")
    nc.sync.dma_start(
        out=bt[:],
        in_=bits[b, px0 : px0 + SB_PX].rearrange("(p w) -> p w", p=P),
    )

    if not do_elem and not do_red:
        return {"F": F, "Fv": Fv, "b": b, "px0": px0}

    # ---- lm1 = 2**bits - 1 exactly: (bits+127)<<23 bitcast f32, -1 ----
    lvl_i = ppool.tile([P, wpp], I32, tag="lvl_i")
    nc.vector.tensor_scalar_add(lvl_i[:], bt[:], 127)
    nc.vector.tensor_scalar(lvl_i[:], lvl_i[:], 23, None, AL.logical_shift_left)
    lm1 = ppool.tile([P, wpp], F32, tag="lm1")
    nc.vector.tensor_scalar_add(lm1[:], lvl_i[:].bitcast(F32), -1.0)

    # ---- channel min/max trees (DVE, 2x f32 stt) ----
    fmax = ppool.tile([P, wpp], F32, tag="fmax")
    fmin = ppool.tile([P, wpp], F32, tag="fmin")
    if do_red:
        _minmax_tree(nc, spool, Fv, fmax, AL.max, wpp)
        _minmax_tree(nc, spool, Fv, fmin, AL.min, wpp)
    else:
        nc.vector.memset(fmax[:], 1.0)
        nc.vector.memset(fmin[:], 0.0)

    if not do_elem:
        return {"F": F, "Fv": Fv, "b": b, "px0": px0}

    # ---- per-pixel params ([P, wpp] tiles, all DVE smalls) ----
    rng = ppool.tile([P, wpp], F32, tag="rng")
    # (fmax + 1e-30) - fmin : div-by-zero guard exact for any real rng
    nc.vector.scalar_tensor_tensor(rng[:], fmax[:], 1e-30, fmin[:], AL.add, AL.subtract)
    rinv = ppool.tile([P, wpp], F32, tag="rinv")
    nc.vector.reciprocal(rinv[:], rng[:])
    u = ppool.tile([P, wpp], F32, tag="u")
    nc.vector.scalar_tensor_tensor(u[:], lm1[:], 0.0, rinv[:], AL.add, AL.mult)
    c2 = ppool.tile([P, wpp], F32, tag="c2")
    nc.vector.scalar_tensor_tensor(c2[:], u[:], -1.0, fmin[:], AL.mult, AL.mult)
    ilm1 = ppool.tile([P, wpp], F32, tag="ilm1")
    nc.vector.reciprocal(ilm1[:], lm1[:])
    v = ppool.tile([P, wpp], F32, tag="v")
    nc.vector.scalar_tensor_tensor(v[:], rng[:], 0.0, ilm1[:], AL.add, AL.mult)
    # bf16 copies of v / fmin for the 4x tail (ACT engine)
    ACTF = mybir.ActivationFunctionType
    vb = ppool.tile([P, wpp], BF16, tag="vb")
    nc.scalar.activation(vb[:], v[:], ACTF.Copy, bias=0.0, scale=1.0)
    fminb = ppool.tile([P, wpp], BF16, tag="fminb")
    nc.scalar.activation(fminb[:], fmin[:], ACTF.Copy, bias=0.0, scale=1.0)

    def bcast(t, dt=F32):
        return t[:].rearrange("p (o w) -> p o w", o=1).to_broadcast((P, C, wpp))

    # ---- big f32 passes, in-place on F (DVE stt, 2x) ----
    # T = F * u
    nc.vector.scalar_tensor_tensor(Fv, Fv, 0.0, bcast(u), AL.add, AL.mult)
    # T2 = T + c2
    nc.vector.scalar_tensor_tensor(Fv, Fv, 0.0, bcast(c2), AL.add, AL.add)
    # ---- rounding on ACT: +M (f32, in place), then -M -> bf16 (exact) ----
    nc.scalar.activation(F[:], F[:], ACTF.Copy, bias=M_MAGIC, scale=1.0)
    Fb = fpool.tile([P, C * wpp], BF16, tag="Fb", bufs=fb_bufs)
    nc.scalar.activation(Fb[:], F[:], ACTF.Copy, bias=-M_MAGIC, scale=1.0)
    return {"Fb": Fb, "vb": vb, "fminb": fminb, "b": b, "px0": px0}


def _back(nc, st, out, do_elem, wpp, timed):
    """Post-round bf16 tail (4x stts) + store."""
    SB_PX = P * wpp
    b, px0 = st["b"], st["px0"]
    if not do_elem:
        # bisection variants: ship F back out untouched (f32)
        Fv = st["Fv"]
        for cc in range(0, C, CCH):
            dst = out[b, cc : cc + CCH, px0 : px0 + SB_PX]
            dst = dst.rearrange("c (p w) -> p c w", p=P)
            nc.sync.dma_start(out=dst, in_=Fv[:, cc : cc + CCH, :])
        return
    Fb = st["Fb"]
    Fbv = Fb[:].rearrange("p (c w) -> p c w", c=C)

    def bcast(t):
        return t[:].rearrange("p (o w) -> p o w", o=1).to_broadcast((P, C, wpp))

    # q = r * v ; o = q + fmin  (plain tensor_tensor: bf16 packed runs 2x;
    # scalar_tensor_tensor would NOT engage the fast mode)
    nc.vector.tensor_tensor(Fbv, Fbv, bcast(st["vb"]), AL.mult)
    nc.vector.tensor_tensor(Fbv, Fbv, bcast(st["fminb"]), AL.add)
    for cc in range(0, C, CCH):
        dst = out[b, cc : cc + CCH, px0 : px0 + SB_PX]
        dst = dst.rearrange("c (p w) -> p c w", p=P)
        if timed:
            # SWDGE inside For_i miscompiles ("ISA wrong length");
            # proxy with HWDGE bf16->bf16 (out is declared bf16)
            nc.sync.dma_start(out=dst, in_=Fbv[:, cc : cc + CCH, :])
        else:
            # SWDGE upcasts bf16 -> f32 on the way out
            nc.gpsimd.dma_start(out=dst, in_=Fbv[:, cc : cc + CCH, :])


def build(
    reps: int = 1,
    variant: str = "full",
    timed_loop: int = 0,
    wpp: int = None,
    f_bufs: int = None,
) -> bass.Bass:
    """Build the per-core Bass program.

    reps: python-unrolled repetitions of the whole (idempotent) workload.
    variant: full | dma | dma_red | dma_elem (bisection aids).
    timed_loop: if >0, build a timing-only program: internal DRAM tensors
    (no input upload), tiny dummy output, and a hardware For_i loop running
    the workload `timed_loop` times.
    """
    _patch_tile_drain()
    if wpp is None:
        wpp = WPP
    if f_bufs is None:
        f_bufs = F_BUFS
    fb_bufs = FB_BUFS
    n_sb = PX // (P * wpp)
    do_red = variant in ("full", "dma_red")
    do_elem = variant in ("full", "dma_elem")
    nc = bass.Bass()
    if timed_loop:
        feat = nc.dram_tensor("features_i", [B_LOC, C, PX], F32)
        bits = nc.dram_tensor("bits_i", [B_LOC, PX], I32)
        out_dt = BF16 if do_elem else F32
        out = nc.dram_tensor("out_i", [B_LOC, C, PX], out_dt)
        dummy = nc.declare_dram_parameter("out", [1, 128], F32, isOutput=True)
    else:
        feat = nc.declare_dram_parameter(
            "features", [B_LOC, C, PX], F32, isOutput=False
        )
        bits = nc.declare_dram_parameter(
            "bit_allocation", [B_LOC, PX], I32, isOutput=False
        )
        out = nc.declare_dram_parameter("out", [B_LOC, C, PX], F32, isOutput=True)

    with TileContext(nc) as tc:
        with (
            tc.tile_pool(name="fpool", bufs=2) as fpool,
            tc.tile_pool(name="ppool", bufs=2) as ppool,
            tc.tile_pool(name="spool", bufs=1) as spool,
        ):
            loop_cm = tc.For_i(0, timed_loop, 1) if timed_loop else nullcontext()
            with loop_cm:
                for _rep in range(reps):
                    blocks = [
                        (b, s) for b in range(B_LOC) for s in range(n_sb)
                    ]
                    pend = {}
                    for i in range(len(blocks) + 1):
                        if i < len(blocks):
                            b, s = blocks[i]
                            pend[i] = _front(
                                nc, fpool, ppool, spool, feat, bits, b, s,
                                do_red, do_elem, wpp, f_bufs, fb_bufs,
                            )
                        if i >= 1:
                            _back(
                                nc, pend.pop(i - 1), out, do_elem, wpp,
                                bool(timed_loop),
                            )
            if timed_loop:
                dtile = ppool.tile([1, 128], F32, tag="dummy")
                nc.vector.memset(dtile[:], 0.0)
                nc.sync.dma_start(out=dummy[:], in_=dtile[:])
    _split_sync_waits(nc)
    return nc


_nc_cache: dict = {}


def _get_nc(reps: int = 1, variant: str = "full", timed_loop: int = 0) -> bass.Bass:
    key = (reps, variant, timed_loop)
    if key not in _nc_cache:
        _nc_cache[key] = build(reps, variant, timed_loop)
    return _nc_cache[key]


def _in_maps(features: np.ndarray, bit_allocation: np.ndarray):
    f = np.ascontiguousarray(features, dtype=np.float32).reshape(B_FULL, C, PX)
    ba = np.ascontiguousarray(bit_allocation, dtype=np.int32).reshape(B_FULL, PX)
    maps = []
    for i in range(N_CORES):
        b0 = i * B_LOC
        maps.append(
            {
                "features": f[b0 : b0 + B_LOC],
                "bit_allocation": ba[b0 : b0 + B_LOC],
            }
        )
    return maps


def run(
    features: np.ndarray,
    bit_allocation: np.ndarray,
    reps: int = 1,
    variant: str = "full",
):
    nc = _get_nc(reps, variant)
    maps = _in_maps(features, bit_allocation)
    res = run_bass_kernel_spmd(nc, maps, core_ids=list(range(N_CORES)))
    outs = [res.results[i]["out"].reshape(B_LOC, C, H, W) for i in range(N_CORES)]
    return np.concatenate(outs, axis=0)


def run_timed(timed_loop: int, variant: str = "full"):
    """Run the timing-only program (no input upload); returns nothing useful."""
    nc = _get_nc(1, variant, timed_loop)
    maps = [{} for _ in range(N_CORES)]
    run_bass_kernel_spmd(nc, maps, core_ids=list(range(N_CORES)))


def kernel(features: np.ndarray, bit_allocation: np.ndarray) -> np.ndarray:
    return run(features, bit_allocation, reps=1)
